# revision 5
# baseline (speedup 1.0000x reference)
"""MoE (top-4 of 16 experts, SwiGLU FFN) on 8 Trainium2 NeuronCores.

Strategy: expert parallelism. The router (x @ Wr, softmax, top-4) is 0.26% of
the FLOPs and runs on host; tokens are gathered per expert on host (the
"all-to-all dispatch"), each core runs the dense SwiGLU FFN for its 2 experts
on its gathered tokens in bf16 (fp32 PSUM accumulation), and the host
scatter-adds the weighted expert outputs back ("combine").

v2 layout (vs the first working version):
  * Warmup matmuls on scratch SBUF run while the first DMAs land, so the PE
    never idles at startup and the HAM clock-gate ramps to 2.4 GHz during the
    DMA wait instead of during real work.
  * Startup DMAs are split across both HWDGE queues (sync + scalar) in
    need-order; all mid-stream input loads stay off the ACT engine until its
    silu work has slack.
  * Stage A is d-outer / chunk-inner within a <=2-chunk "pass", so one weight
    tile serves consecutive matmuls; stage B is f-outer / dd-inner so one h
    tile serves 2 matmuls.
  * Stage-B PSUM->SBUF copies are split between ACT (with fused cw scale) and
    DVE (tensor_scalar_mul with per-partition cw), keeping either engine off
    the critical path of PSUM bank recycling.

Shapes (hardcoded): B=4, S=1024, D=1024, E=16, F=512, TOPK=4. N = B*S = 4096.
All DRAM arrays are pre-tiled on host so every DMA is partition-contiguous.
"""

import numpy as np
import ml_dtypes

import concourse.bass as bass
import concourse.bacc as bacc
import concourse.tile as tile
from concourse import bass_utils, mybir

B, S, D = 4, 1024, 1024
E, F, TOPK = 16, 512, 4
N = B * S
NCORES = 8
EPC = E // NCORES  # experts per core
P = 128
DT = D // P  # 8
FT = F // P  # 4
TCH = 512    # max token chunk (PSUM bank = 512 fp32)
NWARM = 8    # warmup matmuls (~3.4us at cold clock = one HAM window)

BF16 = ml_dtypes.bfloat16

_program_cache: dict[tuple, object] = {}


# ---------------------------------------------------------------- host router
def _route(xf: np.ndarray, Wr: np.ndarray):
    """Top-4 expert ids + renormalized weights per token.

    Renormalized top-k softmax weights == softmax over just the top-k logits,
    so the full softmax denominator is never needed.
    """
    logits = xf @ Wr  # [N, E] fp32
    idx = np.argpartition(-logits, TOPK - 1, axis=1)[:, :TOPK]  # [N, K]
    lt = np.take_along_axis(logits, idx, axis=1)
    lt = lt - lt.max(axis=1, keepdims=True)
    ex = np.exp(lt)
    w = ex / ex.sum(axis=1, keepdims=True)
    return idx, w.astype(np.float32)


def _r128(v):
    return max(P, int(-(-v // P)) * P)


def _chunks_of(C):
    """Even split of C tokens into ceil(C/512) chunks (no alignment needed:
    stage-A matmuls take arbitrary free-dim slices; stage B runs on its own
    128-token grid)."""
    n = -(-C // TCH)
    per, rem = divmod(C, n)
    sizes = [per + 1] * rem + [per] * (n - rem)
    out, t0 = [], 0
    for sz in sizes:
        out.append((t0, sz))
        t0 += sz
    return out


def _passes_of(C):
    """Chunks grouped into passes of <=2 so stage A holds <=4 PSUM banks."""
    chs = _chunks_of(C)
    return [chs[i : i + 2] for i in range(0, len(chs), 2)]


# ---------------------------------------------------------------- device code
def _build_program(caps: tuple):
    """One SPMD program: EPC expert slots with capacities caps[s].

    Inputs (per core), all pre-tiled partition-major on host:
      xt [sum_s 4*128*2*Tpad_s] bf16  tokens, transposed, d-pair-major:
                                      block (s,dp) is [128][i*Tpad+t] with
                                      value X[tok_t, (2dp+i)*128+p]
      wg [EPC, 2, 128, 2048]    bf16  wg[s,fp,p,(i*8+d)*128+q]
                                        = Wg_slot[d*128+p, (2fp+i)*128+q]
      wu [EPC, 2, 128, 2048]    bf16
      wd [EPC, 128, FT, D]      bf16  wd[s, p, t, d] = Wd_slot[t*128+p, d]
      cw [128, CTOT//128]       f32   combine weight per gathered token
    Output:
      y  [CTOT//128, 128, D]    bf16  cw * (silu(x@wg) * (x@wu)) @ wd
    """
    lcaps = [_r128(C) for C in caps]
    CTOT = sum(lcaps)
    slot_passes = [_passes_of(C) for C in caps]
    # flat xt layout: per slot, 4 d-pair blocks of [128, 2*Tpad]
    xt_offs, xoff = [], 0
    for s in range(EPC):
        offs = []
        for dp in range(4):
            offs.append(xoff)
            xoff += P * 2 * lcaps[s]
        xt_offs.append(offs)
    XTELEMS = xoff

    nc = bacc.Bacc("TRN2", target_bir_lowering=False, debug=False)
    bf = mybir.dt.bfloat16
    f32 = mybir.dt.float32

    xt = nc.declare_dram_parameter("xt", [XTELEMS], bf, isOutput=False)
    wg = nc.declare_dram_parameter("wg", [EPC, 2, P, 2 * DT * P], bf, isOutput=False)
    wu = nc.declare_dram_parameter("wu", [EPC, 2, P, 2 * DT * P], bf, isOutput=False)
    wd = nc.declare_dram_parameter("wd", [EPC, P, FT, D], bf, isOutput=False)
    cw = nc.declare_dram_parameter("cw", [P, CTOT // P], f32, isOutput=False)
    y = nc.declare_dram_parameter("y", [CTOT // P, P, D], bf, isOutput=True)

    with tile.TileContext(nc) as tc:
        # -------- warmup: keep PE busy + ramp HAM while startup DMAs land.
        # Own pools so the PSUM bank is released back before the main pools
        # open (8-bank budget is exactly used below).
        with (
            tc.tile_pool(name="warm", bufs=1) as warm,
            tc.tile_pool(name="warmps", bufs=1, space="PSUM") as warmps,
        ):
            wsrc = warm.tile([P, P + TCH], bf, tag="wsrc")
            nc.vector.memset(wsrc[:], 0)
            wps = warmps.tile([P, TCH], f32, tag="wps")
            for _ in range(NWARM):
                nc.tensor.matmul(
                    wps[:], lhsT=wsrc[:, :P], rhs=wsrc[:, P:], start=True, stop=True
                )

        with (
            tc.tile_pool(name="wpool", bufs=2) as wpool,
            tc.tile_pool(name="xpool", bufs=2) as xpool,
            tc.tile_pool(name="hpool", bufs=2) as hpool,
            tc.tile_pool(name="sgpool", bufs=4) as sgpool,
            tc.tile_pool(name="ypool", bufs=4) as ypool,
            tc.tile_pool(name="cwpool", bufs=1) as cwpool,
            tc.tile_pool(name="psA", bufs=4, space="PSUM") as psA,
            tc.tile_pool(name="psB", bufs=4, space="PSUM") as psB,
        ):
            wg_sb = [None] * EPC
            wu_sb = [None] * EPC
            wd_sb = [None] * EPC
            xt_sb = [[None] * 4 for _ in range(EPC)]
            h_sb = [[None] * FT for _ in range(EPC)]

            def load_wgu(eng, which, s, fp):
                dst = wg_sb if which == "wg" else wu_sb
                src = wg if which == "wg" else wu
                if dst[s] is None:
                    dst[s] = [None, None]
                t = wpool.tile([P, 2, DT, P], bf, tag=f"{which}{fp}", name=f"{which}{fp}")
                eng.dma_start(t[:], src[s, fp])
                dst[s][fp] = t

            def load_wd(eng, s):
                wd_sb[s] = wpool.tile([P, FT, D], bf, tag="wd", name="wd")
                eng.dma_start(wd_sb[s][:], wd[s])

            def load_xt(eng, s, dp):
                t = xpool.tile([P, 2, lcaps[s]], bf, tag=f"xt{dp}", name=f"xt{dp}")
                src = xt[
                    xt_offs[s][dp] : xt_offs[s][dp] + P * 2 * lcaps[s]
                ].rearrange("(p x) -> p x", p=P)
                eng.dma_start(t[:], src)
                xt_sb[s][dp] = t

            # -------- startup DMA issues (need-ordered per HWDGE queue).
            # sync queue: first token d-pairs, then wu (needed after G of f0),
            # then all of slot 1's tokens.
            load_xt(nc.sync, 0, 0)
            load_xt(nc.sync, 0, 1)
            # scalar queue: first f-pair of wg (G of f0/f1), remaining tokens.
            load_wgu(nc.scalar, "wg", 0, 0)
            load_xt(nc.scalar, 0, 2)
            load_xt(nc.scalar, 0, 3)
            load_wgu(nc.scalar, "wg", 0, 1)
            load_wgu(nc.sync, "wu", 0, 0)
            load_wgu(nc.sync, "wu", 0, 1)
            for dp in range(4):
                load_xt(nc.sync, 1, dp)
            cw_sb = cwpool.tile([P, CTOT // P], f32, tag="cw")
            nc.gpsimd.dma_start(cw_sb[:], cw[:, :])

            # mid-stream loads: (slot, f-phase-end) -> list of issue thunks.
            # All on the scalar engine, placed between its silu groups.
            deferred = {
                (0, 0): [lambda: load_wd(nc.scalar, 0)],
                (0, 1): [lambda: load_wgu(nc.scalar, "wg", 1, 0)],
                (0, 2): [lambda: load_wgu(nc.scalar, "wg", 1, 1)],
                (0, 3): [lambda: load_wgu(nc.scalar, "wu", 1, 0)],
                (0, "b1"): [lambda: load_wgu(nc.scalar, "wu", 1, 1)],
                (0, "b4"): [lambda: load_wd(nc.scalar, 1)],
            }

            off = 0  # global token offset (cw / y rows), 128-aligned per slot
            for s in range(EPC):
                Cs = caps[s]
                for pchunks in slot_passes[s]:
                    for f in range(FT):
                        fp, fi = divmod(f, 2)
                        # G: h_g = wg[f].T @ x, [F-part, tok-free]
                        psg, sgt = {}, {}
                        for d in range(DT):
                            dp, di = divmod(d, 2)
                            wsl = wg_sb[s][fp][:, fi, d, :]
                            for ci, (t0, tch) in enumerate(pchunks):
                                if d == 0:
                                    psg[ci] = psA.tile([P, TCH], f32, tag="ps", name="psg")
                                nc.tensor.matmul(
                                    psg[ci][:, :tch],
                                    lhsT=wsl,
                                    rhs=xt_sb[s][dp][:, di, t0 : t0 + tch],
                                    start=(d == 0),
                                    stop=(d == DT - 1),
                                )
                        for ci, (t0, tch) in enumerate(pchunks):
                            sgt[ci] = sgpool.tile([P, TCH], f32, tag="sg", name="sg")
                            nc.scalar.activation(
                                sgt[ci][:, :tch],
                                psg[ci][:, :tch],
                                mybir.ActivationFunctionType.Silu,
                            )
                        # U: h_u = wu[f].T @ x; h = silu(h_g) * h_u
                        psu = {}
                        for d in range(DT):
                            dp, di = divmod(d, 2)
                            wsl = wu_sb[s][fp][:, fi, d, :]
                            for ci, (t0, tch) in enumerate(pchunks):
                                if d == 0:
                                    psu[ci] = psA.tile([P, TCH], f32, tag="ps", name="psu")
                                nc.tensor.matmul(
                                    psu[ci][:, :tch],
                                    lhsT=wsl,
                                    rhs=xt_sb[s][dp][:, di, t0 : t0 + tch],
                                    start=(d == 0),
                                    stop=(d == DT - 1),
                                )
                        for ci, (t0, tch) in enumerate(pchunks):
                            if h_sb[s][f] is None:
                                h_sb[s][f] = hpool.tile(
                                    [P, lcaps[s]], bf, tag=f"h{f}", name=f"h{f}"
                                )
                            nc.vector.tensor_mul(
                                out=h_sb[s][f][:, t0 : t0 + tch],
                                in0=sgt[ci][:, :tch],
                                in1=psu[ci][:, :tch],
                            )
                        for fn in deferred.pop((s, f), []) if (
                            pchunks is slot_passes[s][0]
                        ) else []:
                            fn()
                # -------- stage B: y[m] = cw * h.T @ wd, [tok-part, D-free]
                NM = -(-Cs // P)
                for m in range(NM):
                    mr = min(P, Cs - m * P)
                    cc = off // P + m
                    y_sb = ypool.tile([P, D], bf, tag="y")
                    py0 = psB.tile([P, TCH], f32, tag="py")
                    py1 = psB.tile([P, TCH], f32, tag="py")
                    for f in range(FT):
                        lh = h_sb[s][f][:, m * P : m * P + mr]
                        nc.tensor.matmul(
                            py0[:mr],
                            lhsT=lh,
                            rhs=wd_sb[s][:, f, :TCH],
                            start=(f == 0),
                            stop=(f == FT - 1),
                        )
                        nc.tensor.matmul(
                            py1[:mr],
                            lhsT=lh,
                            rhs=wd_sb[s][:, f, TCH:],
                            start=(f == 0),
                            stop=(f == FT - 1),
                        )
                    cwc = cw_sb[:mr, cc : cc + 1]
                    nc.scalar.activation(
                        y_sb[:mr, :TCH],
                        py0[:mr],
                        mybir.ActivationFunctionType.Copy,
                        scale=cwc,
                    )
                    nc.vector.tensor_scalar_mul(y_sb[:mr, TCH:], py1[:mr], cwc)
                    nc.gpsimd.dma_start(y[cc, :mr], y_sb[:mr])
                    for fn in deferred.pop((s, f"b{m}"), []):
                        fn()
                off += lcaps[s]
    nc.compile()
    return nc


def _get_program(caps):
    if caps not in _program_cache:
        _program_cache[caps] = _build_program(caps)
    return _program_cache[caps]


# ------------------------------------------------------------------ profiling
def _ensure_ntff_hook():
    """The container's `antenv` stub lacks `axon_hooks`, so trn_boot's NTFF
    profile hook never gets registered and trace=True degrades to no-op.
    Register the module + ctypes hook at runtime."""
    import sys
    import types

    import antenv

    if "antenv.axon_hooks" not in sys.modules:
        mod = types.ModuleType("antenv.axon_hooks")
        mod._hook = None

        def set_axon_ntff_profile_hook(h):
            mod._hook = h

        def get_axon_ntff_profile_hook():
            return mod._hook

        mod.set_axon_ntff_profile_hook = set_axon_ntff_profile_hook
        mod.get_axon_ntff_profile_hook = get_axon_ntff_profile_hook
        sys.modules["antenv.axon_hooks"] = mod
        antenv.axon_hooks = mod
    mod = sys.modules["antenv.axon_hooks"]
    if mod._hook is None:
        from trn_agent_boot.trn_boot import _ntff_profile_via_ctypes

        mod.set_axon_ntff_profile_hook(
            _ntff_profile_via_ctypes("/opt/axon/libaxon_pjrt.so")
        )


# ---------------------------------------------------------------- entry point
def _run(inputs: dict, trace: bool = False, trace_all: bool = False):
    x = np.asarray(inputs["x"], dtype=np.float32)
    Wr = np.asarray(inputs["Wr"], dtype=np.float32)
    Wg = np.asarray(inputs["Wg"], dtype=np.float32)
    Wu = np.asarray(inputs["Wu"], dtype=np.float32)
    Wd = np.asarray(inputs["Wd"], dtype=np.float32)

    xf = x.reshape(N, D)
    idx, w = _route(xf, Wr)

    # group (token, weight) by expert
    flat_e = idx.ravel()
    flat_t = np.repeat(np.arange(N, dtype=np.int64), TOPK)
    flat_w = w.ravel()
    order = np.argsort(flat_e, kind="stable")
    ge, gt, gw = flat_e[order], flat_t[order], flat_w[order]
    counts = np.bincount(ge, minlength=E)
    starts = np.zeros(E + 1, dtype=np.int64)
    np.cumsum(counts, out=starts[1:])

    # global pairing: sort experts by count desc, core c gets ranks (c, 15-c);
    # slot 0 holds the larger one. Minimizes both slot capacities.
    by_size = sorted(range(E), key=lambda e: -counts[e])
    slot_experts = [
        [by_size[c], by_size[E - 1 - c]] for c in range(NCORES)
    ]  # [core][slot] -> expert id
    caps = tuple(
        int(max(counts[slot_experts[c][s]] for c in range(NCORES)))
        for s in range(EPC)
    )
    lcaps = [_r128(Cs) for Cs in caps]
    CTOT = sum(lcaps)
    slot_off = np.cumsum([0] + list(lcaps))

    XTELEMS = sum(4 * P * 2 * lc for lc in lcaps)
    xt_all = np.zeros((NCORES, XTELEMS), dtype=BF16)
    cw_all = np.zeros((NCORES, P, CTOT // P), dtype=np.float32)
    wg_all = np.zeros((NCORES, EPC, 2, P, 2 * DT * P), dtype=BF16)
    wu_all = np.zeros((NCORES, EPC, 2, P, 2 * DT * P), dtype=BF16)
    wd_all = np.zeros((NCORES, EPC, P, FT, D), dtype=BF16)

    def wgu_tiles(W):  # [D, F] -> [2, 128, 2048]
        a = W.astype(BF16).reshape(DT, P, FT, P)  # [d, p, f, q]
        return (
            a.transpose(2, 1, 0, 3)  # [f, p, d, q]
            .reshape(2, 2, P, DT, P)  # [fp, i, p, d, q]
            .transpose(0, 2, 1, 3, 4)
            .reshape(2, P, 2 * DT * P)
        )

    tok_lists = {}
    for c in range(NCORES):
        xoff = 0
        for s in range(EPC):
            e = slot_experts[c][s]
            toks = gt[starts[e] : starts[e + 1]]
            tok_lists[(c, s)] = toks
            ne = len(toks)
            Tpad = lcaps[s]
            xs = np.zeros((Tpad, D), dtype=BF16)
            xs[:ne] = xf[toks].astype(BF16)
            a = xs.reshape(Tpad, DT, P).transpose(1, 2, 0)  # [d, p, t]
            b = (
                a.reshape(4, 2, P, Tpad)
                .transpose(0, 2, 1, 3)
                .reshape(4, P, 2 * Tpad)
            )
            nb = 4 * P * 2 * Tpad
            xt_all[c, xoff : xoff + nb] = b.ravel()
            xoff += nb
            cw_flat = np.zeros(Tpad, dtype=np.float32)
            cw_flat[:ne] = gw[starts[e] : starts[e + 1]]
            cw_all[c, :, slot_off[s] // P : slot_off[s + 1] // P] = (
                cw_flat.reshape(-1, P).T
            )
            wg_all[c, s] = wgu_tiles(Wg[e])
            wu_all[c, s] = wgu_tiles(Wu[e])
            wd_all[c, s] = Wd[e].astype(BF16).reshape(FT, P, D).transpose(1, 0, 2)

    nc = _get_program(caps)
    in_maps = [
        {
            "xt": xt_all[c],
            "wg": wg_all[c],
            "wu": wu_all[c],
            "wd": wd_all[c],
            "cw": cw_all[c],
        }
        for c in range(NCORES)
    ]
    kwargs = {}
    if trace:
        _ensure_ntff_hook()
        kwargs = dict(trace=True)
        if trace_all:
            kwargs["trace_cores"] = list(range(NCORES))
    res = bass_utils.run_bass_kernel_spmd(
        nc, in_maps, core_ids=list(range(NCORES)), **kwargs
    )

    out = np.zeros((N, D), dtype=np.float32)
    for c in range(NCORES):
        yc = res.results[c]["y"].reshape(CTOT, D)
        for s in range(EPC):
            toks = tok_lists[(c, s)]
            out[toks] += yc[slot_off[s] : slot_off[s] + len(toks)].astype(
                np.float32
            )
    return out.reshape(B, S, D), res.exec_time_ns


# Pre-register the NTFF hook shim at import: if the grading harness sets
# BASS_TRACE=1, run_bass_kernel_spmd's axon trace path imports
# antenv.axon_hooks, which the container's antenv stub lacks.
try:
    _ensure_ntff_hook()
except Exception:
    pass


def kernel(**inputs) -> np.ndarray:
    out, _ = _run(inputs, trace=False)
    return out


# revision 6
# speedup vs baseline: 1.0393x; 1.0393x over previous
"""MoE (top-4 of 16 experts, SwiGLU FFN) on 8 Trainium2 NeuronCores.

Strategy: expert parallelism. The router (x @ Wr, softmax, top-4) is 0.26% of
the FLOPs and runs on host; tokens are gathered per expert on host (the
"all-to-all dispatch"), each core runs the dense SwiGLU FFN for its 2 experts
on its gathered tokens in bf16 (fp32 PSUM accumulation), and the host
scatter-adds the weighted expert outputs back ("combine").

v2 layout (vs the first working version):
  * Warmup matmuls on scratch SBUF run while the first DMAs land, so the PE
    never idles at startup and the HAM clock-gate ramps to 2.4 GHz during the
    DMA wait instead of during real work.
  * Startup DMAs are split across both HWDGE queues (sync + scalar) in
    need-order; all mid-stream input loads stay off the ACT engine until its
    silu work has slack.
  * Stage A is d-outer / chunk-inner within a <=2-chunk "pass", so one weight
    tile serves consecutive matmuls; stage B is f-outer / dd-inner so one h
    tile serves 2 matmuls.
  * Stage-B PSUM->SBUF copies are split between ACT (with fused cw scale) and
    DVE (tensor_scalar_mul with per-partition cw), keeping either engine off
    the critical path of PSUM bank recycling.

Shapes (hardcoded): B=4, S=1024, D=1024, E=16, F=512, TOPK=4. N = B*S = 4096.
All DRAM arrays are pre-tiled on host so every DMA is partition-contiguous.
"""

import numpy as np
import ml_dtypes

import concourse.bass as bass
import concourse.bacc as bacc
import concourse.tile as tile
from concourse import bass_utils, mybir

B, S, D = 4, 1024, 1024
E, F, TOPK = 16, 512, 4
N = B * S
NCORES = 8
EPC = E // NCORES  # experts per core
P = 128
DT = D // P  # 8
FT = F // P  # 4
TCH = 512    # max token chunk (PSUM bank = 512 fp32)
NWARM = 8    # warmup matmuls (~3.4us at cold clock = one HAM window)

BF16 = ml_dtypes.bfloat16

_program_cache: dict[tuple, object] = {}


# ---------------------------------------------------------------- host router
def _route(xf: np.ndarray, Wr: np.ndarray):
    """Top-4 expert ids + renormalized weights per token.

    Renormalized top-k softmax weights == softmax over just the top-k logits,
    so the full softmax denominator is never needed.
    """
    logits = xf @ Wr  # [N, E] fp32
    idx = np.argpartition(-logits, TOPK - 1, axis=1)[:, :TOPK]  # [N, K]
    lt = np.take_along_axis(logits, idx, axis=1)
    lt = lt - lt.max(axis=1, keepdims=True)
    ex = np.exp(lt)
    w = ex / ex.sum(axis=1, keepdims=True)
    return idx, w.astype(np.float32)


def _r128(v):
    return max(P, int(-(-v // P)) * P)


def _chunks_of(C):
    """Even split of C tokens into ceil(C/512) chunks (no alignment needed:
    stage-A matmuls take arbitrary free-dim slices; stage B runs on its own
    128-token grid)."""
    n = -(-C // TCH)
    per, rem = divmod(C, n)
    sizes = [per + 1] * rem + [per] * (n - rem)
    out, t0 = [], 0
    for sz in sizes:
        out.append((t0, sz))
        t0 += sz
    return out


def _passes_of(C):
    """Chunks grouped into passes of <=2 so stage A holds <=4 PSUM banks."""
    chs = _chunks_of(C)
    return [chs[i : i + 2] for i in range(0, len(chs), 2)]


# ---------------------------------------------------------------- device code
DORDER = (0, 2, 4, 6, 1, 3, 5, 7)  # matches startup DMA arrival order


def _build_program(caps: tuple):
    """One SPMD program: EPC expert slots with capacities caps[s].

    Inputs (per core), all pre-tiled partition-major on host:
      xt [sum_s 8*128*Tpad_s] bf16  tokens, transposed, d-major:
                                    block (s,d) is [128][t] with value
                                    X[tok_t, d*128+p]
      wg [EPC, FT, 128, DT*128] bf16  wg[s,f,p,d*128+q] = Wg_slot[d*128+p, f*128+q]
      wu [EPC, FT, 128, DT*128] bf16
      wd [EPC, 128, FT, D]      bf16  wd[s, p, t, d] = Wd_slot[t*128+p, d]
      cw [128, CTOT//128]       f32   combine weight per gathered token
    Output:
      y  [CTOT//128, 128, D]    bf16  cw * (silu(x@wg) * (x@wu)) @ wd
    """
    lcaps = [_r128(C) for C in caps]
    CTOT = sum(lcaps)
    slot_passes = [_passes_of(C) for C in caps]
    # flat xt layout: per slot, 8 per-d blocks of [128, Tpad]
    xt_offs, xoff = [], 0
    for s in range(EPC):
        offs = []
        for d in range(DT):
            offs.append(xoff)
            xoff += P * lcaps[s]
        xt_offs.append(offs)
    XTELEMS = xoff

    nc = bacc.Bacc("TRN2", target_bir_lowering=False, debug=False)
    bf = mybir.dt.bfloat16
    f32 = mybir.dt.float32

    xt = nc.declare_dram_parameter("xt", [XTELEMS], bf, isOutput=False)
    wg = nc.declare_dram_parameter("wg", [EPC, FT, P, DT * P], bf, isOutput=False)
    wu = nc.declare_dram_parameter("wu", [EPC, FT, P, DT * P], bf, isOutput=False)
    wd = nc.declare_dram_parameter("wd", [EPC, P, FT, D], bf, isOutput=False)
    cw = nc.declare_dram_parameter("cw", [P, CTOT // P], f32, isOutput=False)
    y = nc.declare_dram_parameter("y", [CTOT // P, P, D], bf, isOutput=True)

    with tile.TileContext(nc) as tc:
        with (
            tc.tile_pool(name="warm", bufs=1) as warm,
            tc.tile_pool(name="wpool", bufs=2) as wpool,
            tc.tile_pool(name="xpool", bufs=2) as xpool,
            tc.tile_pool(name="hpool", bufs=2) as hpool,
            tc.tile_pool(name="sgpool", bufs=4) as sgpool,
            tc.tile_pool(name="ypool", bufs=4) as ypool,
            tc.tile_pool(name="cwpool", bufs=1) as cwpool,
            tc.tile_pool(name="psA", bufs=4, space="PSUM") as psA,
            tc.tile_pool(name="psB", bufs=4, space="PSUM") as psB,
        ):
            # -------- warmup: keep PE busy + ramp the HAM clock-gate while
            # the startup DMAs land. The scratch pool stays open for the whole
            # program so no later tile aliases it (an aliased tile would
            # inherit a WAR dependency on all warmup matmuls and its DMA
            # would wait ~4us). PSUM scratch comes from psA's ring: the
            # ring just starts one slot shifted.
            wsrc = warm.tile([P, P + TCH], bf, tag="wsrc")
            nc.gpsimd.memset(wsrc[:], 0)
            wps = psA.tile([P, TCH], f32, tag="ps", name="wps")
            for _ in range(NWARM):
                nc.tensor.matmul(
                    wps[:], lhsT=wsrc[:, :P], rhs=wsrc[:, P:], start=True, stop=True
                )

            wg_sb = [[None] * FT for _ in range(EPC)]
            wu_sb = [[None] * FT for _ in range(EPC)]
            wd_sb = [None] * EPC
            xt_sb = [[None] * DT for _ in range(EPC)]
            h_sb = [[None] * FT for _ in range(EPC)]

            def load_wgu(eng, which, s, f):
                dst = wg_sb if which == "wg" else wu_sb
                src = wg if which == "wg" else wu
                t = wpool.tile(
                    [P, DT, P], bf, tag=f"{which}{f}", name=f"{which}{f}"
                )
                eng.dma_start(t[:], src[s, f])
                dst[s][f] = t

            def load_wd(eng, s):
                wd_sb[s] = wpool.tile([P, FT, D], bf, tag="wd", name="wd")
                eng.dma_start(wd_sb[s][:], wd[s])

            def load_xt(eng, s, d):
                t = xpool.tile([P, lcaps[s]], bf, tag=f"xt{d}", name=f"xt{d}")
                src = xt[
                    xt_offs[s][d] : xt_offs[s][d] + P * lcaps[s]
                ].rearrange("(p x) -> p x", p=P)
                eng.dma_start(t[:], src)
                xt_sb[s][d] = t

            # -------- startup DMA issues, interleaved across the two HWDGE
            # queues in the order the PE will need the data (DORDER).
            load_xt(nc.sync, 0, 0)
            load_wgu(nc.scalar, "wg", 0, 0)
            load_xt(nc.sync, 0, 2)
            load_wgu(nc.scalar, "wg", 0, 1)
            load_xt(nc.sync, 0, 4)
            load_xt(nc.scalar, 0, 1)
            load_xt(nc.sync, 0, 6)
            load_xt(nc.scalar, 0, 3)
            load_wgu(nc.sync, "wu", 0, 0)
            load_xt(nc.scalar, 0, 5)
            load_wgu(nc.sync, "wu", 0, 1)
            load_xt(nc.scalar, 0, 7)
            load_wgu(nc.sync, "wu", 0, 2)
            load_wgu(nc.scalar, "wg", 0, 2)
            load_wgu(nc.sync, "wu", 0, 3)
            load_wgu(nc.scalar, "wg", 0, 3)
            cw_sb = cwpool.tile([P, CTOT // P], f32, tag="cw")
            nc.gpsimd.dma_start(cw_sb[:], cw[:, :])

            # mid-stream loads, fired at (slot, marker) points of the build.
            deferred = {
                (0, "p0f0"): [
                    lambda: load_wd(nc.scalar, 0),
                    lambda: load_xt(nc.sync, 1, 0),
                    lambda: load_xt(nc.sync, 1, 2),
                ],
                (0, "p0f1"): [
                    lambda: load_wgu(nc.scalar, "wg", 1, 0),
                    lambda: load_wgu(nc.scalar, "wg", 1, 1),
                    lambda: load_xt(nc.sync, 1, 4),
                    lambda: load_xt(nc.sync, 1, 6),
                ],
                (0, "p1f0"): [
                    lambda: load_xt(nc.scalar, 1, 1),
                    lambda: load_xt(nc.scalar, 1, 3),
                    lambda: load_wgu(nc.sync, "wu", 1, 0),
                    lambda: load_wgu(nc.sync, "wu", 1, 1),
                ],
                (0, "p1f1"): [
                    lambda: load_xt(nc.scalar, 1, 5),
                    lambda: load_xt(nc.scalar, 1, 7),
                    lambda: load_wgu(nc.sync, "wu", 1, 2),
                    lambda: load_wgu(nc.sync, "wu", 1, 3),
                ],
                (0, "b1"): [
                    lambda: load_wgu(nc.scalar, "wg", 1, 2),
                    lambda: load_wgu(nc.scalar, "wg", 1, 3),
                ],
                (0, "b4"): [lambda: load_wd(nc.scalar, 1)],
            }

            off = 0  # global token offset (cw / y rows), 128-aligned per slot
            for s in range(EPC):
                Cs = caps[s]
                for pi, pchunks in enumerate(slot_passes[s]):
                    for fp in range(2):
                        fs = (2 * fp, 2 * fp + 1)
                        # G: h_g = wg[f].T @ x, [F-part, tok-free]
                        psg, sgt, psu = {}, {}, {}
                        for di, d in enumerate(DORDER):
                            for f in fs:
                                wsl = wg_sb[s][f][:, d, :]
                                for ci, (t0, tch) in enumerate(pchunks):
                                    if di == 0:
                                        psg[(f, ci)] = psA.tile(
                                            [P, TCH], f32, tag="ps", name="psg"
                                        )
                                    nc.tensor.matmul(
                                        psg[(f, ci)][:, :tch],
                                        lhsT=wsl,
                                        rhs=xt_sb[s][d][:, t0 : t0 + tch],
                                        start=(di == 0),
                                        stop=(di == DT - 1),
                                    )
                        for f in fs:
                            for ci, (t0, tch) in enumerate(pchunks):
                                sgt[(f, ci)] = sgpool.tile(
                                    [P, TCH], f32, tag="sg", name="sg"
                                )
                                nc.scalar.activation(
                                    sgt[(f, ci)][:, :tch],
                                    psg[(f, ci)][:, :tch],
                                    mybir.ActivationFunctionType.Silu,
                                )
                        # U: h_u = wu[f].T @ x; h = silu(h_g) * h_u
                        for di, d in enumerate(DORDER):
                            for f in fs:
                                wsl = wu_sb[s][f][:, d, :]
                                for ci, (t0, tch) in enumerate(pchunks):
                                    if di == 0:
                                        psu[(f, ci)] = psA.tile(
                                            [P, TCH], f32, tag="ps", name="psu"
                                        )
                                    nc.tensor.matmul(
                                        psu[(f, ci)][:, :tch],
                                        lhsT=wsl,
                                        rhs=xt_sb[s][d][:, t0 : t0 + tch],
                                        start=(di == 0),
                                        stop=(di == DT - 1),
                                    )
                        for f in fs:
                            for ci, (t0, tch) in enumerate(pchunks):
                                if h_sb[s][f] is None:
                                    h_sb[s][f] = hpool.tile(
                                        [P, lcaps[s]], bf, tag=f"h{f}", name=f"h{f}"
                                    )
                                nc.vector.tensor_mul(
                                    out=h_sb[s][f][:, t0 : t0 + tch],
                                    in0=sgt[(f, ci)][:, :tch],
                                    in1=psu[(f, ci)][:, :tch],
                                )
                        for fn in deferred.pop((s, f"p{pi}f{fp}"), []):
                            fn()
                # -------- stage B: y[m] = cw * h.T @ wd, [tok-part, D-free]
                NM = -(-Cs // P)
                for m in range(NM):
                    mr = min(P, Cs - m * P)
                    cc = off // P + m
                    y_sb = ypool.tile([P, D], bf, tag="y", name="y_sb")
                    py0 = psB.tile([P, TCH], f32, tag="py", name="py0")
                    py1 = psB.tile([P, TCH], f32, tag="py", name="py1")
                    for f in range(FT):
                        lh = h_sb[s][f][:, m * P : m * P + mr]
                        nc.tensor.matmul(
                            py0[:mr],
                            lhsT=lh,
                            rhs=wd_sb[s][:, f, :TCH],
                            start=(f == 0),
                            stop=(f == FT - 1),
                        )
                        nc.tensor.matmul(
                            py1[:mr],
                            lhsT=lh,
                            rhs=wd_sb[s][:, f, TCH:],
                            start=(f == 0),
                            stop=(f == FT - 1),
                        )
                    cwc = cw_sb[:mr, cc : cc + 1]
                    nc.scalar.activation(
                        y_sb[:mr, :TCH],
                        py0[:mr],
                        mybir.ActivationFunctionType.Copy,
                        scale=cwc,
                    )
                    nc.vector.tensor_scalar_mul(y_sb[:mr, TCH:], py1[:mr], cwc)
                    nc.gpsimd.dma_start(y[cc, :mr], y_sb[:mr])
                    for fn in deferred.pop((s, f"b{m}"), []):
                        fn()
                off += lcaps[s]
    nc.compile()
    return nc


def _get_program(caps):
    if caps not in _program_cache:
        _program_cache[caps] = _build_program(caps)
    return _program_cache[caps]


# ------------------------------------------------------------------ profiling
def _ensure_ntff_hook():
    """The container's `antenv` stub lacks `axon_hooks`, so trn_boot's NTFF
    profile hook never gets registered and trace=True degrades to no-op.
    Register the module + ctypes hook at runtime."""
    import sys
    import types

    import antenv

    if "antenv.axon_hooks" not in sys.modules:
        mod = types.ModuleType("antenv.axon_hooks")
        mod._hook = None

        def set_axon_ntff_profile_hook(h):
            mod._hook = h

        def get_axon_ntff_profile_hook():
            return mod._hook

        mod.set_axon_ntff_profile_hook = set_axon_ntff_profile_hook
        mod.get_axon_ntff_profile_hook = get_axon_ntff_profile_hook
        sys.modules["antenv.axon_hooks"] = mod
        antenv.axon_hooks = mod
    mod = sys.modules["antenv.axon_hooks"]
    if mod._hook is None:
        from trn_agent_boot.trn_boot import _ntff_profile_via_ctypes

        mod.set_axon_ntff_profile_hook(
            _ntff_profile_via_ctypes("/opt/axon/libaxon_pjrt.so")
        )


# ---------------------------------------------------------------- entry point
def _run(inputs: dict, trace: bool = False, trace_all: bool = False):
    x = np.asarray(inputs["x"], dtype=np.float32)
    Wr = np.asarray(inputs["Wr"], dtype=np.float32)
    Wg = np.asarray(inputs["Wg"], dtype=np.float32)
    Wu = np.asarray(inputs["Wu"], dtype=np.float32)
    Wd = np.asarray(inputs["Wd"], dtype=np.float32)

    xf = x.reshape(N, D)
    idx, w = _route(xf, Wr)

    # group (token, weight) by expert
    flat_e = idx.ravel()
    flat_t = np.repeat(np.arange(N, dtype=np.int64), TOPK)
    flat_w = w.ravel()
    order = np.argsort(flat_e, kind="stable")
    ge, gt, gw = flat_e[order], flat_t[order], flat_w[order]
    counts = np.bincount(ge, minlength=E)
    starts = np.zeros(E + 1, dtype=np.int64)
    np.cumsum(counts, out=starts[1:])

    # global pairing: sort experts by count desc, core c gets ranks (c, 15-c);
    # slot 0 holds the larger one. Minimizes both slot capacities.
    by_size = sorted(range(E), key=lambda e: -counts[e])
    slot_experts = [
        [by_size[c], by_size[E - 1 - c]] for c in range(NCORES)
    ]  # [core][slot] -> expert id
    caps = tuple(
        int(max(counts[slot_experts[c][s]] for c in range(NCORES)))
        for s in range(EPC)
    )
    lcaps = [_r128(Cs) for Cs in caps]
    CTOT = sum(lcaps)
    slot_off = np.cumsum([0] + list(lcaps))

    XTELEMS = sum(DT * P * lc for lc in lcaps)
    xt_all = np.zeros((NCORES, XTELEMS), dtype=BF16)
    cw_all = np.zeros((NCORES, P, CTOT // P), dtype=np.float32)
    wg_all = np.zeros((NCORES, EPC, FT, P, DT * P), dtype=BF16)
    wu_all = np.zeros((NCORES, EPC, FT, P, DT * P), dtype=BF16)
    wd_all = np.zeros((NCORES, EPC, P, FT, D), dtype=BF16)

    def wgu_tiles(W):  # [D, F] -> [FT, 128, DT*128]
        a = W.astype(BF16).reshape(DT, P, FT, P)  # [d, p, f, q]
        return a.transpose(2, 1, 0, 3).reshape(FT, P, DT * P)

    tok_lists = {}
    for c in range(NCORES):
        xoff = 0
        for s in range(EPC):
            e = slot_experts[c][s]
            toks = gt[starts[e] : starts[e + 1]]
            tok_lists[(c, s)] = toks
            ne = len(toks)
            Tpad = lcaps[s]
            xs = np.zeros((Tpad, D), dtype=BF16)
            xs[:ne] = xf[toks].astype(BF16)
            a = xs.reshape(Tpad, DT, P).transpose(1, 2, 0)  # [d, p, t]
            nb = DT * P * Tpad
            xt_all[c, xoff : xoff + nb] = a.ravel()
            xoff += nb
            cw_flat = np.zeros(Tpad, dtype=np.float32)
            cw_flat[:ne] = gw[starts[e] : starts[e + 1]]
            cw_all[c, :, slot_off[s] // P : slot_off[s + 1] // P] = (
                cw_flat.reshape(-1, P).T
            )
            wg_all[c, s] = wgu_tiles(Wg[e])
            wu_all[c, s] = wgu_tiles(Wu[e])
            wd_all[c, s] = Wd[e].astype(BF16).reshape(FT, P, D).transpose(1, 0, 2)

    nc = _get_program(caps)
    in_maps = [
        {
            "xt": xt_all[c],
            "wg": wg_all[c],
            "wu": wu_all[c],
            "wd": wd_all[c],
            "cw": cw_all[c],
        }
        for c in range(NCORES)
    ]
    kwargs = {}
    if trace:
        _ensure_ntff_hook()
        kwargs = dict(trace=True)
        if trace_all:
            kwargs["trace_cores"] = list(range(NCORES))
    res = bass_utils.run_bass_kernel_spmd(
        nc, in_maps, core_ids=list(range(NCORES)), **kwargs
    )

    out = np.zeros((N, D), dtype=np.float32)
    for c in range(NCORES):
        yc = res.results[c]["y"].reshape(CTOT, D)
        for s in range(EPC):
            toks = tok_lists[(c, s)]
            out[toks] += yc[slot_off[s] : slot_off[s] + len(toks)].astype(
                np.float32
            )
    return out.reshape(B, S, D), res.exec_time_ns


# Pre-register the NTFF hook shim at import: if the grading harness sets
# BASS_TRACE=1, run_bass_kernel_spmd's axon trace path imports
# antenv.axon_hooks, which the container's antenv stub lacks.
try:
    _ensure_ntff_hook()
except Exception:
    pass


def kernel(**inputs) -> np.ndarray:
    out, _ = _run(inputs, trace=False)
    return out
